# revision 1
# baseline (speedup 1.0000x reference)
"""Trainium2 Bass kernel for nn_EnergyToRateConverter.

Computes Eyring rates  fwd = pref*exp(-(bar - G_from)/RT),
rev = reversible ? pref*exp(-(bar - G_to)/RT) : 0  for B=1M batch rows.

Strategy (pure data parallel over 8 cores, batch split 8 ways):
  * Host transposes inputs into "feature-major" layout X = [state.T;
    barrier.T] of shape (80, B) so that the per-transition gather
    G_from/G_to and the barrier subtraction become one small constant
    matmul W.T @ X with contraction over SBUF partitions:
        W[s, j]    = 1  for s == from_idx[j] (fwd cols) / to_idx[j] (rev)
        W[32+j, j] = -1 (subtract barrier j)
    Output columns are [48 fwd | reversible rev | pad-to-16]; rates for
    non-reversible transitions are never computed — the device output
    buffer is pre-zeroed, so those rows are just never written.
  * 80 and the padded M are multiples of 16, which is what the HWDGE
    descriptor->SDMA-engine split needs to use all 16 engines.
  * X is shipped as an fp16 hi + fp8e3m4 lo pair (3 bytes/element, 25%
    less than f32); the two matmul passes accumulate in PSUM, recovering
    ~5e-4 worst-case relative accuracy at one PE cycle per row each.
  * ScalarE evaluates out = exp(x*inv_rt + ln(pref)) straight from PSUM.
  * Input DMAs ride the SP HWDGE ring, output DMAs the ACT ring, so
    output waits never head-of-line-block input prefetch.
"""

import os

import ml_dtypes
import numpy as np

B = 1048576
N_CORES = 8
BC = B // N_CORES  # 131072 batch rows per core
NS = 32
NT = 48
K = NS + NT  # 80 contraction rows: states then barriers

F_SUPER = 4096  # batch columns per DMA super-tile
F_PSUM = 2048  # batch columns per PSUM tile / ACT op
F_MM = 512  # batch columns per matmul (one PSUM bank)

T = 298.15
K_B = 1.380649e-23
H = 6.62607015e-34
R = 0.008314462618
EYRING_PREFACTOR = K_B * T / H
RT = R * T
INV_RT = float(np.float32(1.0 / RT))  # reference casts 1/RT to f32
LN_PREF = float(np.log(EYRING_PREFACTOR))
LO_SCALE = 64.0

_cached = {}


def _build_program(m_out):
    from concourse import bacc, mybir
    from concourse.tile import TileContext

    nc = bacc.Bacc(
        None, target_bir_lowering=False, debug=False, num_devices=N_CORES
    )
    xh = nc.dram_tensor("x_hi", [K, BC], mybir.dt.float16, kind="ExternalInput")
    xl = nc.dram_tensor("x_lo", [K, BC], mybir.dt.float8e3, kind="ExternalInput")
    wh = nc.dram_tensor("w_hi", [K, m_out], mybir.dt.float16, kind="ExternalInput")
    wl = nc.dram_tensor("w_lo", [K, m_out], mybir.dt.float8e3, kind="ExternalInput")
    y = nc.dram_tensor("y", [m_out, BC], mybir.dt.float32, kind="ExternalOutput")

    exp = mybir.ActivationFunctionType.Exp

    with TileContext(nc) as tc:
        with (
            tc.tile_pool(name="consts", bufs=1) as cpool,
            tc.tile_pool(name="inp", bufs=6) as ipool,
            tc.tile_pool(name="outp", bufs=4) as opool,
            tc.tile_pool(name="psum", bufs=2, space="PSUM") as ppool,
        ):
            wth = cpool.tile([K, m_out], mybir.dt.float16)
            nc.sync.dma_start(wth[:], wh[:])
            wtl = cpool.tile([K, m_out], mybir.dt.float8e3)
            nc.sync.dma_start(wtl[:], wl[:])
            bias_t = cpool.tile([128, 1], mybir.dt.float32)
            nc.vector.memset(bias_t[:], LN_PREF)

            def supertile(c0, width, ip, op, tg, fp=None):
                fp = fp or F_PSUM
                hi = ip.tile([K, width], mybir.dt.float16, name=f"hi{tg}", tag=f"hi{tg}")
                nc.sync.dma_start(hi[:], xh[:, c0 : c0 + width])
                lo = ip.tile([K, width], mybir.dt.float8e3, name=f"lo{tg}", tag=f"lo{tg}")
                nc.gpsimd.dma_start(lo[:], xl[:, c0 : c0 + width])
                out = op.tile(
                    [m_out, width], mybir.dt.float32, name=f"out{tg}", tag=f"out{tg}"
                )
                for p in range(width // fp):
                    ps = ppool.tile([m_out, fp], mybir.dt.float32, name="ps", tag="ps")
                    for m in range(fp // F_MM):
                        a = p * fp + m * F_MM
                        s = slice(m * F_MM, (m + 1) * F_MM)
                        nc.tensor.matmul(
                            ps[:, s], wth[:], hi[:, a : a + F_MM],
                            start=True, stop=False,
                        )
                        nc.tensor.matmul(
                            ps[:, s], wtl[:], lo[:, a : a + F_MM],
                            start=False, stop=True,
                        )
                    po = slice(p * fp, (p + 1) * fp)
                    nc.scalar.activation(
                        out[:, po], ps[:],
                        exp, bias=bias_t[:m_out], scale=INV_RT,
                    )
                    eng = nc.scalar if (c0 // F_PSUM + p) % 2 == 0 else nc.sync
                    eng.dma_start(
                        y[:, c0 + p * fp : c0 + (p + 1) * fp], out[:, po]
                    )

            if BC % F_SUPER == 0 and BC >= 4 * F_SUPER and F_SUPER == 2 * F_PSUM:
                edge_fp = max(F_PSUM // 2, F_MM)
                supertile(0, F_PSUM, ipool, opool, "", fp=edge_fp)
                for t in range(1, BC // F_SUPER):
                    supertile((t - 1) * F_SUPER + F_PSUM, F_SUPER, ipool, opool, "")
                supertile(BC - F_PSUM, F_PSUM, ipool, opool, "", fp=edge_fp)
            else:
                for t in range(BC // F_SUPER):
                    supertile(t * F_SUPER, F_SUPER, ipool, opool, "")
    nc.compile()
    return nc


def _host_prep(state_energies, barrier_energies, from_idx, to_idx, reversible):
    se = np.asarray(state_energies, dtype=np.float32)
    be = np.asarray(barrier_energies, dtype=np.float32)
    fi = np.asarray(from_idx).astype(np.int64)
    ti = np.asarray(to_idx).astype(np.int64)
    rv = np.asarray(reversible).astype(bool)

    x = np.empty((K, B), np.float32)
    x[0:NS] = se.T
    x[NS:] = be.T
    xh = x.astype(np.float16)
    # residual scaled by 64 (folded back via w_lo = w/64) to stay in
    # fp8e3m4's normal range
    xl = ((x - xh.astype(np.float32)) * np.float32(LO_SCALE)).astype(
        ml_dtypes.float8_e3m4
    )

    rev_idx = np.flatnonzero(rv)  # transitions with a reverse rate
    n_rev = len(rev_idx)
    m_out = ((NT + n_rev + 15) // 16) * 16

    w = np.zeros((K, m_out), np.float32)
    cols = np.arange(NT)
    w[fi, cols] = 1.0
    w[NS + cols, cols] = -1.0
    rcols = NT + np.arange(n_rev)
    w[ti[rev_idx], rcols] = 1.0
    w[NS + rev_idx, rcols] = -1.0
    wb_hi = w.astype(np.float16)
    wb_lo = (w / np.float32(LO_SCALE)).astype(ml_dtypes.float8_e3m4)
    return xh, xl, wb_hi, wb_lo, rev_idx, m_out


last_results = None


def kernel(state_energies, barrier_energies, from_idx, to_idx, reversible):
    global last_results
    from concourse.bass_utils import run_bass_kernel_spmd

    xh, xl, wb_hi, wb_lo, rev_idx, m_out = _host_prep(
        state_energies, barrier_energies, from_idx, to_idx, reversible
    )

    if m_out not in _cached:
        _cached[m_out] = _build_program(m_out)
    nc = _cached[m_out]

    in_maps = []
    for c in range(N_CORES):
        sl = slice(c * BC, (c + 1) * BC)
        in_maps.append(
            {
                "x_hi": np.ascontiguousarray(xh[:, sl]),
                "x_lo": np.ascontiguousarray(xl[:, sl]),
                "w_hi": wb_hi,
                "w_lo": wb_lo,
            }
        )

    res = run_bass_kernel_spmd(
        nc,
        in_maps,
        core_ids=list(range(N_CORES)),
        trace=bool(int(os.environ.get("KERNEL_TRACE", "0"))),
    )
    last_results = res

    n_rev = len(rev_idx)
    forward = np.empty((B, NT), np.float32)
    reverse = np.zeros((B, NT), np.float32)
    for c, r in enumerate(res.results):
        yc = r["y"]
        forward[c * BC : (c + 1) * BC] = yc[:NT].T
        reverse[c * BC : (c + 1) * BC, rev_idx] = yc[NT : NT + n_rev].T
    return forward, reverse



# revision 4
# speedup vs baseline: 1.1099x; 1.1099x over previous
"""Trainium2 Bass kernel for nn_EnergyToRateConverter.

Computes Eyring rates  fwd = pref*exp(-(bar - G_from)/RT),
rev = reversible ? pref*exp(-(bar - G_to)/RT) : 0  for B=1M batch rows.

Strategy (pure data parallel over 8 cores, batch split 8 ways):
  * Host transposes inputs into one feature-major fp16 tensor
    X = [state.T; (barrier - C).T] of shape (80, B).  Subtracting the
    barrier mean C (~40) first puts barriers in the same fp16 binade as
    the state energies, so a single fp16 pass already hits ~1.3e-2
    worst-case relative error (gate is 2e-2) without a second
    residual-correction matmul pass.
  * One constant matmul W.T @ X per 512-column chunk fuses the
    per-transition gather AND the barrier subtraction:
        W[from_idx[j], j] = 1 (fwd cols) / W[to_idx[j], j] = 1 (rev)
        W[32+j, j] = -1  (subtract barrier j)
    Output rows are [48 fwd | n_rev rev] with no padding; rates for
    non-reversible transitions are never computed.
  * ScalarE evaluates out = exp(psum*inv_rt + (ln(pref) - C*inv_rt))
    straight from PSUM, writing bf16 (exponent range of f32, 2^-9
    rounding) — halving output DMA bytes vs f32.
  * Input DMAs alternate the SP and Pool HWDGE queues, output DMAs ride
    the otherwise-idle DVE queue, so the ACT sequencer only runs the
    activations and no queue head-of-line-blocks another.
"""

import os

import numpy as np

B = 1048576
N_CORES = 8
BC = B // N_CORES  # 131072 batch rows per core
NS = 32
NT = 48
K = NS + NT  # 80 contraction rows: states then shifted barriers

F_SUPER = 8192  # batch columns per DMA super-tile (16KB/partition fp16)
F_PSUM = 2048  # batch columns per PSUM tile / ACT op (4 banks)
F_MM = 512  # batch columns per matmul (PE moving-dim max, one bank)

T = 298.15
K_B = 1.380649e-23
H = 6.62607015e-34
R = 0.008314462618
EYRING_PREFACTOR = K_B * T / H
RT = R * T
INV_RT = float(np.float32(1.0 / RT))  # reference casts 1/RT to f32
LN_PREF = float(np.log(EYRING_PREFACTOR))

_cached = {}


def _build_program(m_out, bias_val):
    from concourse import bacc, mybir
    from concourse.tile import TileContext

    nc = bacc.Bacc(
        None, target_bir_lowering=False, debug=False, num_devices=N_CORES
    )
    x = nc.dram_tensor("x", [K, BC], mybir.dt.float16, kind="ExternalInput")
    w = nc.dram_tensor("w", [K, m_out], mybir.dt.float16, kind="ExternalInput")
    y = nc.dram_tensor("y", [m_out, BC], mybir.dt.bfloat16, kind="ExternalOutput")

    exp = mybir.ActivationFunctionType.Exp

    with TileContext(nc) as tc:
        with (
            tc.tile_pool(name="consts", bufs=1) as cpool,
            tc.tile_pool(name="inp", bufs=3) as ipool,
            tc.tile_pool(name="outp", bufs=3) as opool,
            tc.tile_pool(name="psum", bufs=2, space="PSUM") as ppool,
        ):
            wt = cpool.tile([K, m_out], mybir.dt.float16)
            nc.sync.dma_start(wt[:], w[:])
            bias_t = cpool.tile([128, 1], mybir.dt.float32)
            nc.vector.memset(bias_t[:], bias_val)

            for t in range(BC // F_SUPER):
                c0 = t * F_SUPER
                xt = ipool.tile([K, F_SUPER], mybir.dt.float16, name="xt", tag="xt")
                nc.gpsimd.dma_start(xt[:], x[:, c0 : c0 + F_SUPER])
                out = opool.tile(
                    [m_out, F_SUPER], mybir.dt.bfloat16, name="out", tag="out"
                )
                for p in range(F_SUPER // F_PSUM):
                    ps = ppool.tile([m_out, F_PSUM], mybir.dt.float32, name="ps", tag="ps")
                    for m in range(F_PSUM // F_MM):
                        a = p * F_PSUM + m * F_MM
                        s = slice(m * F_MM, (m + 1) * F_MM)
                        nc.tensor.matmul(
                            ps[:, s], wt[:], xt[:, a : a + F_MM],
                            start=True, stop=True,
                        )
                    po = slice(p * F_PSUM, (p + 1) * F_PSUM)
                    nc.scalar.activation(
                        out[:, po], ps[:],
                        exp, bias=bias_t[:m_out], scale=INV_RT,
                    )
                nc.sync.dma_start(y[:, c0 : c0 + F_SUPER], out[:])
    nc.compile()
    return nc


def _host_prep(state_energies, barrier_energies, from_idx, to_idx, reversible):
    se = np.asarray(state_energies, dtype=np.float32)
    be = np.asarray(barrier_energies, dtype=np.float32)
    fi = np.asarray(from_idx).astype(np.int64)
    ti = np.asarray(to_idx).astype(np.int64)
    rv = np.asarray(reversible).astype(bool)

    # Shift barriers by their (rounded) mean so fp16 keeps ~4 more
    # absolute bits; folded back exactly through the activation bias.
    c_shift = float(np.round(np.float64(be[:4096].mean())))

    x = np.empty((K, B), np.float16)
    x[0:NS] = se.T
    x[NS:] = (be - np.float32(c_shift)).T

    rev_idx = np.flatnonzero(rv)  # transitions with a reverse rate
    n_rev = len(rev_idx)
    m_out = NT + n_rev

    w = np.zeros((K, m_out), np.float16)
    cols = np.arange(NT)
    w[fi, cols] = 1.0
    w[NS + cols, cols] = -1.0
    if n_rev:
        rcols = NT + np.arange(n_rev)
        w[ti[rev_idx], rcols] = 1.0
        w[NS + rev_idx, rcols] = -1.0
    bias_val = LN_PREF - c_shift * INV_RT
    return x, w, rev_idx, m_out, bias_val


last_results = None


def kernel(state_energies, barrier_energies, from_idx, to_idx, reversible):
    global last_results
    from concourse.bass_utils import run_bass_kernel_spmd

    x, w, rev_idx, m_out, bias_val = _host_prep(
        state_energies, barrier_energies, from_idx, to_idx, reversible
    )

    key = (m_out, bias_val)
    if key not in _cached:
        _cached[key] = _build_program(m_out, bias_val)
    nc = _cached[key]

    in_maps = []
    for c in range(N_CORES):
        sl = slice(c * BC, (c + 1) * BC)
        in_maps.append({"x": np.ascontiguousarray(x[:, sl]), "w": w})

    res = run_bass_kernel_spmd(
        nc,
        in_maps,
        core_ids=list(range(N_CORES)),
        trace=bool(int(os.environ.get("KERNEL_TRACE", "0"))),
    )
    last_results = res

    n_rev = len(rev_idx)
    forward = np.empty((B, NT), np.float32)
    reverse = np.zeros((B, NT), np.float32)
    for c, r in enumerate(res.results):
        yc = np.asarray(r["y"])
        # bf16 -> f32 via bit shift (exact, faster than astype)
        yf = (yc.view(np.uint16).astype(np.uint32) << 16).view(np.float32)
        forward[c * BC : (c + 1) * BC] = yf[:NT].T
        if n_rev:
            reverse[c * BC : (c + 1) * BC][:, rev_idx] = yf[NT:].T
    return forward, reverse
